# revision 17
# baseline (speedup 1.0000x reference)
"""AttentionBasedAdapter Trainium2 kernel.

Data-parallel over the batch dim: 8 NeuronCores, 2 batches (2048 tokens)
per core; the (small) k/v context projections are precomputed on host in
fp32 and replicated to all cores; the heavy attention compute runs on
device in bf16 (fp32 PSUM accumulate).

Device math per core (tokens flattened, feature-on-partition layout so
no on-device transposes are needed):
    qT  = Wq^T @ xT, scaled by 1/sqrt(A), + bq*scale     [A, tok]
    eT  = exp(kT_chunks^T @ qT)   stored [c, tok] chunks
    oUT = v^T-chunks @ eT         unnormalized           [A, tok]
    den = 1^T @ (sum of eT chunks)                       [1, tok]
    outT = (Wo^T @ oUT) * (1/den bcast) + bo             [P, tok]
The 1/den softmax normalization commutes past the Wo projection (it is a
per-token row scaling), and attn rows summing to 1 makes the host-folded
k/v biases exact.
"""

from contextlib import ExitStack

import ml_dtypes
import numpy as np

import concourse.bass as bass
import concourse.tile as tile
from concourse import bacc, bass_utils, mybir

BF16 = ml_dtypes.bfloat16

B, T, D = 16, 1024, 512
C, DC, A, P = 4096, 512, 512, 512
NCORE = 8
BPC = B // NCORE            # batches per core
TOK = BPC * T               # 2048 tokens per core
GRP = 512                   # tokens processed per group
NG = TOK // GRP             # 4 groups
KD = D // 128               # 4 contraction chunks of 128
NCC = C // 128              # 32 context chunks of 128
SCALE = float(1.0 / np.sqrt(A))

F32 = mybir.dt.float32
BF = mybir.dt.bfloat16


def _build():
    nc = bacc.Bacc(
        "TRN2",
        target_bir_lowering=False,
        debug=False,
        enable_asserts=False,
        num_devices=NCORE,
    )
    xT = nc.dram_tensor("xT", [D, TOK], BF, kind="ExternalInput").ap()
    kTd = nc.dram_tensor("kT", [A, C], BF, kind="ExternalInput").ap()
    vmd = nc.dram_tensor("vm", [C, A], BF, kind="ExternalInput").ap()
    wq = nc.dram_tensor("wq", [D, A], BF, kind="ExternalInput").ap()
    wo = nc.dram_tensor("wo", [A, P], BF, kind="ExternalInput").ap()
    bqs = nc.dram_tensor("bqs", [A], F32, kind="ExternalInput").ap()
    boe = nc.dram_tensor("boe", [P], F32, kind="ExternalInput").ap()
    onesf = nc.dram_tensor("onesf", [128, 1], F32, kind="ExternalInput").ap()
    outT = nc.dram_tensor("outT", [P, TOK], F32, kind="ExternalOutput").ap()

    with tile.TileContext(nc) as tc:
        with ExitStack() as ctx:
            consts = ctx.enter_context(tc.tile_pool(name="consts", bufs=1))
            big = ctx.enter_context(tc.tile_pool(name="big", bufs=1))
            sc = ctx.enter_context(tc.tile_pool(name="sc", bufs=2))
            xq = ctx.enter_context(tc.tile_pool(name="xq", bufs=2))
            ot_pool = ctx.enter_context(tc.tile_pool(name="ot", bufs=2))
            ds_pool = ctx.enter_context(tc.tile_pool(name="ds", bufs=2))
            zs_pool = ctx.enter_context(tc.tile_pool(name="zs", bufs=2))
            rd_pool = ctx.enter_context(tc.tile_pool(name="rd", bufs=2))
            ps = ctx.enter_context(tc.tile_pool(name="ps", bufs=6, space="PSUM"))
            psd = ctx.enter_context(tc.tile_pool(name="psd", bufs=2, space="PSUM"))
            dram = ctx.enter_context(tc.tile_pool(name="dram", bufs=2, space="DRAM"))

            # ---- constants / weights in SBUF ----
            onesf_sb = consts.tile([128, 1], F32, tag="onesf")
            nc.sync.dma_start(onesf_sb[:], onesf[:])
            w_sb = {}
            for name, ap in (("wq", wq), ("wo", wo)):
                t = consts.tile([128, KD, 512], BF, tag=name)
                nc.sync.dma_start(t[:], ap.rearrange("(k p) a -> p k a", p=128))
                w_sb[name] = t
            bq_sb = consts.tile([128, KD], F32, tag="bqs")
            nc.sync.dma_start(bq_sb[:], bqs.rearrange("(m p) -> p m", p=128))
            bo_sb = consts.tile([128, KD], F32, tag="boe")
            nc.sync.dma_start(bo_sb[:], boe.rearrange("(m p) -> p m", p=128))

            # The sync (SP) HWDGE ring is FIFO: issue the first token-group's
            # xT before the 4 MB kT so the q projection can start immediately.
            # v goes on the scalar (ACT) HWDGE ring, in parallel with kT.
            xT_r = xT.rearrange("(k p) t -> p k t", p=128)
            xT0_sb = xq.tile([128, KD, GRP], BF, tag="xT")
            nc.sync.dma_start(xT0_sb[:], xT_r[:, :, 0:GRP])

            kT_sb = big.tile([128, KD, C], BF, tag="kT")
            nc.sync.dma_start(kT_sb[:], kTd.rearrange("(m p) c -> p m c", p=128))
            v_sb = big.tile([128, NCC, 512], BF, tag="v")
            nc.scalar.dma_start(v_sb[:], vmd.rearrange("(i p) a -> p i a", p=128))

            for g in range(NG):
                # ---- load x^T for this token group ----
                if g == 0:
                    xT_sb = xT0_sb
                else:
                    xT_sb = xq.tile([128, KD, GRP], BF, tag="xT")
                    nc.sync.dma_start(xT_sb[:], xT_r[:, :, GRP * g : GRP * (g + 1)])

                # ---- q^T[a, tok] = Wq^T @ x^T, scaled, + bq*scale ----
                qT_sb = xq.tile([128, KD, GRP], BF, tag="qT")
                for m in range(KD):
                    pq = ps.tile([128, GRP], F32, tag="mm")
                    for k in range(KD):
                        nc.tensor.matmul(
                            pq[:],
                            w_sb["wq"][:, k, 128 * m : 128 * (m + 1)],
                            xT_sb[:, k, :],
                            start=(k == 0),
                            stop=(k == KD - 1),
                        )
                    nc.scalar.activation(
                        qT_sb[:, m, :],
                        pq[:],
                        mybir.ActivationFunctionType.Identity,
                        bias=bq_sb[:, m : m + 1],
                        scale=SCALE,
                    )

                # ---- scores^T chunks + exp; dsum accumulates exp chunks ----
                expT_sb = sc.tile([128, NCC, GRP], BF, tag="sc")
                dsum = ds_pool.tile([128, GRP], F32, tag="dsum")
                for i in range(NCC):
                    pscr = ps.tile([128, GRP], F32, tag="mm")
                    for m in range(KD):
                        nc.tensor.matmul(
                            pscr[:],
                            kT_sb[:, m, 128 * i : 128 * (i + 1)],
                            qT_sb[:, m, :],
                            start=(m == 0),
                            stop=(m == KD - 1),
                        )
                    nc.scalar.activation(
                        expT_sb[:, i, :], pscr[:], mybir.ActivationFunctionType.Exp
                    )
                    if i == 1:
                        nc.vector.tensor_add(
                            dsum[:], expT_sb[:, 0, :], expT_sb[:, 1, :]
                        )
                    elif i > 1:
                        nc.vector.tensor_add(dsum[:], dsum[:], expT_sb[:, i, :])

                # ---- den = 1^T @ dsum ; rden broadcast to all partitions ----
                pden = psd.tile([1, GRP], F32, tag="den")
                nc.tensor.matmul(pden[:], onesf_sb[:], dsum[:], start=True, stop=True)
                rden_row = rd_pool.tile([1, GRP], F32, tag="rdenr")
                nc.vector.reciprocal(rden_row[:], pden[:])
                rden_dr = dram.tile([1, GRP], F32, tag="rdendr")
                nc.sync.dma_start(rden_dr[:], rden_row[:])
                rden_bc = rd_pool.tile([128, GRP], F32, tag="rdenb")
                nc.sync.dma_start(rden_bc[:], rden_dr[:].partition_broadcast(128))

                # ---- oUT[a, tok] = v^T-chunks @ eT (unnormalized) ----
                oT_sb = ot_pool.tile([128, KD, GRP], BF, tag="oT")
                for m in range(KD):
                    po = ps.tile([128, GRP], F32, tag="mm")
                    for i in range(NCC):
                        nc.tensor.matmul(
                            po[:],
                            v_sb[:, i, 128 * m : 128 * (m + 1)],
                            expT_sb[:, i, :],
                            start=(i == 0),
                            stop=(i == NCC - 1),
                        )
                    nc.scalar.copy(oT_sb[:, m, :], po[:])

                # ---- out^T = (Wo^T @ oUT) * rden + bo ----
                for m in range(KD):
                    pz = ps.tile([128, GRP], F32, tag="mm")
                    for k in range(KD):
                        nc.tensor.matmul(
                            pz[:],
                            w_sb["wo"][:, k, 128 * m : 128 * (m + 1)],
                            oT_sb[:, k, :],
                            start=(k == 0),
                            stop=(k == KD - 1),
                        )
                    zt = zs_pool.tile([128, GRP], F32, tag="ztmp")
                    nc.vector.tensor_mul(zt[:], pz[:], rden_bc[:])
                    zs = zs_pool.tile([128, GRP], F32, tag="zs")
                    nc.scalar.activation(
                        zs[:],
                        zt[:],
                        mybir.ActivationFunctionType.Identity,
                        bias=bo_sb[:, m : m + 1],
                    )
                    nc.sync.dma_start(
                        outT[128 * m : 128 * (m + 1), GRP * g : GRP * (g + 1)], zs[:]
                    )

    nc.compile()
    return nc


_CACHE = {}


def _get_nc():
    if "nc" not in _CACHE:
        _CACHE["nc"] = _build()
    return _CACHE["nc"]


def _prepare_in_maps(inputs):
    return _make_in_maps(**inputs)


def _make_in_maps(model_embed, context_embed, Wq, bq, Wk, bk, Wv, bv, Wo, bo):
    model_embed = np.asarray(model_embed, dtype=np.float32)
    context_embed = np.asarray(context_embed, dtype=np.float32)
    Wq, bq = np.asarray(Wq, np.float32), np.asarray(bq, np.float32)
    Wk, bk = np.asarray(Wk, np.float32), np.asarray(bk, np.float32)
    Wv, bv = np.asarray(Wv, np.float32), np.asarray(bv, np.float32)
    Wo, bo = np.asarray(Wo, np.float32), np.asarray(bo, np.float32)

    k_h = context_embed @ Wk + bk       # [C, A] fp32
    v_h = context_embed @ Wv + bv       # [C, A] fp32
    shared = {
        "kT": np.ascontiguousarray(k_h.T).astype(BF16),
        "vm": v_h.astype(BF16),
        "wq": Wq.astype(BF16),
        "wo": Wo.astype(BF16),
        "bqs": (bq * SCALE).astype(np.float32),
        "boe": bo.astype(np.float32),
        "onesf": np.ones((128, 1), dtype=np.float32),
    }
    in_maps = []
    for c in range(NCORE):
        xs = model_embed[BPC * c : BPC * (c + 1)].reshape(TOK, D)
        m = dict(shared)
        m["xT"] = np.ascontiguousarray(xs.T).astype(BF16)
        in_maps.append(m)
    return in_maps


def kernel(**inputs):
    nc = _get_nc()
    in_maps = _make_in_maps(**inputs)
    res = bass_utils.run_bass_kernel_spmd(nc, in_maps, core_ids=list(range(NCORE)))

    out = np.empty((B, T, P), dtype=np.float32)
    for c in range(NCORE):
        outT_c = res.results[c]["outT"]  # [P, TOK]
        out[BPC * c : BPC * (c + 1)] = outT_c.T.reshape(BPC, T, P)
    return out


# revision 18
# speedup vs baseline: 1.2173x; 1.2173x over previous
"""AttentionBasedAdapter Trainium2 kernel.

Data-parallel over the batch dim: 8 NeuronCores, 2 batches (2048 tokens)
per core; the (small) k/v context projections are precomputed on host in
fp32 and replicated to all cores; the heavy attention compute runs on
device in bf16 (fp32 PSUM accumulate).

Device math per core (tokens flattened, feature-on-partition layout so
no on-device transposes are needed):
    qT  = Wq^T @ xT, scaled by 1/sqrt(A), + bq*scale     [A, tok]
    eT  = exp(kT_chunks^T @ qT)   stored [c, tok] chunks
    oUT = v^T-chunks @ eT         unnormalized           [A, tok]
    den = 1^T @ (sum of eT chunks)                       [1, tok]
    outT = (Wo^T @ oUT) * (1/den bcast) + bo             [P, tok]
The 1/den softmax normalization commutes past the Wo projection (it is a
per-token row scaling), and attn rows summing to 1 makes the host-folded
k/v biases exact.
"""

from contextlib import ExitStack

import ml_dtypes
import numpy as np

import concourse.bass as bass
import concourse.tile as tile
from concourse import bacc, bass_utils, mybir

BF16 = ml_dtypes.bfloat16

B, T, D = 16, 1024, 512
C, DC, A, P = 4096, 512, 512, 512
NCORE = 8
BPC = B // NCORE            # batches per core
TOK = BPC * T               # 2048 tokens per core
GRP = 512                   # tokens processed per group
NG = TOK // GRP             # 4 groups
KD = D // 128               # 4 contraction chunks of 128
NCC = C // 128              # 32 context chunks of 128
SCALE = float(1.0 / np.sqrt(A))

F32 = mybir.dt.float32
BF = mybir.dt.bfloat16


def _build():
    nc = bacc.Bacc(
        "TRN2",
        target_bir_lowering=False,
        debug=False,
        enable_asserts=False,
        num_devices=NCORE,
    )
    xT = nc.dram_tensor("xT", [D, TOK], BF, kind="ExternalInput").ap()
    kTd = nc.dram_tensor("kT", [A, C], BF, kind="ExternalInput").ap()
    vmd = nc.dram_tensor("vm", [C, A], BF, kind="ExternalInput").ap()
    wq = nc.dram_tensor("wq", [D, A], BF, kind="ExternalInput").ap()
    wo = nc.dram_tensor("wo", [A, P], BF, kind="ExternalInput").ap()
    bqs = nc.dram_tensor("bqs", [A], F32, kind="ExternalInput").ap()
    boe = nc.dram_tensor("boe", [P], F32, kind="ExternalInput").ap()
    onesf = nc.dram_tensor("onesf", [128, 1], F32, kind="ExternalInput").ap()
    outT = nc.dram_tensor("outT", [P, TOK], F32, kind="ExternalOutput").ap()

    with tile.TileContext(nc) as tc:
        with ExitStack() as ctx:
            consts = ctx.enter_context(tc.tile_pool(name="consts", bufs=1))
            big = ctx.enter_context(tc.tile_pool(name="big", bufs=1))
            sc = ctx.enter_context(tc.tile_pool(name="sc", bufs=2))
            xq = ctx.enter_context(tc.tile_pool(name="xq", bufs=2))
            ot_pool = ctx.enter_context(tc.tile_pool(name="ot", bufs=2))
            ds_pool = ctx.enter_context(tc.tile_pool(name="ds", bufs=2))
            zs_pool = ctx.enter_context(tc.tile_pool(name="zs", bufs=2))
            rd_pool = ctx.enter_context(tc.tile_pool(name="rd", bufs=2))
            ps = ctx.enter_context(tc.tile_pool(name="ps", bufs=6, space="PSUM"))
            psd = ctx.enter_context(tc.tile_pool(name="psd", bufs=2, space="PSUM"))
            dram = ctx.enter_context(tc.tile_pool(name="dram", bufs=2, space="DRAM"))

            # ---- constants / weights in SBUF ----
            onesf_sb = consts.tile([128, 1], F32, tag="onesf")
            nc.sync.dma_start(onesf_sb[:], onesf[:])
            w_sb = {}
            for name, ap in (("wq", wq), ("wo", wo)):
                t = consts.tile([128, KD, 512], BF, tag=name)
                nc.sync.dma_start(t[:], ap.rearrange("(k p) a -> p k a", p=128))
                w_sb[name] = t
            bq_sb = consts.tile([128, KD], F32, tag="bqs")
            nc.sync.dma_start(bq_sb[:], bqs.rearrange("(m p) -> p m", p=128))
            bo_sb = consts.tile([128, KD], F32, tag="boe")
            nc.sync.dma_start(bo_sb[:], boe.rearrange("(m p) -> p m", p=128))

            # The sync (SP) HWDGE ring is FIFO: issue the first token-group's
            # xT before the 4 MB kT so the q projection can start immediately.
            # v goes on the scalar (ACT) HWDGE ring, in parallel with kT.
            xT_r = xT.rearrange("(k p) t -> p k t", p=128)
            xT0_sb = xq.tile([128, KD, GRP], BF, tag="xT")
            nc.sync.dma_start(xT0_sb[:], xT_r[:, :, 0:GRP])

            kT_sb = big.tile([128, KD, C], BF, tag="kT")
            nc.sync.dma_start(kT_sb[:], kTd.rearrange("(m p) c -> p m c", p=128))
            v_sb = big.tile([128, NCC, 512], BF, tag="v")
            nc.sync.dma_start(v_sb[:], vmd.rearrange("(i p) a -> p i a", p=128))

            for g in range(NG):
                # ---- load x^T for this token group ----
                if g == 0:
                    xT_sb = xT0_sb
                else:
                    xT_sb = xq.tile([128, KD, GRP], BF, tag="xT")
                    nc.sync.dma_start(xT_sb[:], xT_r[:, :, GRP * g : GRP * (g + 1)])

                # ---- q^T[a, tok] = Wq^T @ x^T, scaled, + bq*scale ----
                qT_sb = xq.tile([128, KD, GRP], BF, tag="qT")
                for m in range(KD):
                    pq = ps.tile([128, GRP], F32, tag="mm")
                    for k in range(KD):
                        nc.tensor.matmul(
                            pq[:],
                            w_sb["wq"][:, k, 128 * m : 128 * (m + 1)],
                            xT_sb[:, k, :],
                            start=(k == 0),
                            stop=(k == KD - 1),
                        )
                    nc.scalar.activation(
                        qT_sb[:, m, :],
                        pq[:],
                        mybir.ActivationFunctionType.Identity,
                        bias=bq_sb[:, m : m + 1],
                        scale=SCALE,
                    )

                # ---- scores^T chunks + exp; dsum accumulates exp chunks ----
                expT_sb = sc.tile([128, NCC, GRP], BF, tag="sc")
                dsum = ds_pool.tile([128, GRP], F32, tag="dsum")
                for i in range(NCC):
                    pscr = ps.tile([128, GRP], F32, tag="mm")
                    for m in range(KD):
                        nc.tensor.matmul(
                            pscr[:],
                            kT_sb[:, m, 128 * i : 128 * (i + 1)],
                            qT_sb[:, m, :],
                            start=(m == 0),
                            stop=(m == KD - 1),
                        )
                    nc.scalar.activation(
                        expT_sb[:, i, :], pscr[:], mybir.ActivationFunctionType.Exp
                    )
                    if i == 1:
                        nc.vector.tensor_add(
                            dsum[:], expT_sb[:, 0, :], expT_sb[:, 1, :]
                        )
                    elif i > 1:
                        nc.vector.tensor_add(dsum[:], dsum[:], expT_sb[:, i, :])

                # ---- den = 1^T @ dsum ; rden broadcast to all partitions ----
                pden = psd.tile([1, GRP], F32, tag="den")
                nc.tensor.matmul(pden[:], onesf_sb[:], dsum[:], start=True, stop=True)
                rden_row = rd_pool.tile([1, GRP], F32, tag="rdenr")
                nc.vector.reciprocal(rden_row[:], pden[:])
                rden_dr = dram.tile([1, GRP], F32, tag="rdendr")
                nc.sync.dma_start(rden_dr[:], rden_row[:])
                rden_bc = rd_pool.tile([128, GRP], F32, tag="rdenb")
                nc.sync.dma_start(rden_bc[:], rden_dr[:].partition_broadcast(128))

                # ---- oUT[a, tok] = v^T-chunks @ eT (unnormalized) ----
                oT_sb = ot_pool.tile([128, KD, GRP], BF, tag="oT")
                for m in range(KD):
                    po = ps.tile([128, GRP], F32, tag="mm")
                    for i in range(NCC):
                        nc.tensor.matmul(
                            po[:],
                            v_sb[:, i, 128 * m : 128 * (m + 1)],
                            expT_sb[:, i, :],
                            start=(i == 0),
                            stop=(i == NCC - 1),
                        )
                    nc.scalar.copy(oT_sb[:, m, :], po[:])

                # ---- out^T = (Wo^T @ oUT) * rden + bo ----
                for m in range(KD):
                    pz = ps.tile([128, GRP], F32, tag="mm")
                    for k in range(KD):
                        nc.tensor.matmul(
                            pz[:],
                            w_sb["wo"][:, k, 128 * m : 128 * (m + 1)],
                            oT_sb[:, k, :],
                            start=(k == 0),
                            stop=(k == KD - 1),
                        )
                    zt = zs_pool.tile([128, GRP], F32, tag="ztmp")
                    nc.vector.tensor_mul(zt[:], pz[:], rden_bc[:])
                    zs = zs_pool.tile([128, GRP], F32, tag="zs")
                    nc.scalar.activation(
                        zs[:],
                        zt[:],
                        mybir.ActivationFunctionType.Identity,
                        bias=bo_sb[:, m : m + 1],
                    )
                    nc.sync.dma_start(
                        outT[128 * m : 128 * (m + 1), GRP * g : GRP * (g + 1)], zs[:]
                    )

    nc.compile()
    return nc


_CACHE = {}


def _get_nc():
    if "nc" not in _CACHE:
        _CACHE["nc"] = _build()
    return _CACHE["nc"]


def _prepare_in_maps(inputs):
    return _make_in_maps(**inputs)


def _make_in_maps(model_embed, context_embed, Wq, bq, Wk, bk, Wv, bv, Wo, bo):
    model_embed = np.asarray(model_embed, dtype=np.float32)
    context_embed = np.asarray(context_embed, dtype=np.float32)
    Wq, bq = np.asarray(Wq, np.float32), np.asarray(bq, np.float32)
    Wk, bk = np.asarray(Wk, np.float32), np.asarray(bk, np.float32)
    Wv, bv = np.asarray(Wv, np.float32), np.asarray(bv, np.float32)
    Wo, bo = np.asarray(Wo, np.float32), np.asarray(bo, np.float32)

    k_h = context_embed @ Wk + bk       # [C, A] fp32
    v_h = context_embed @ Wv + bv       # [C, A] fp32
    shared = {
        "kT": np.ascontiguousarray(k_h.T).astype(BF16),
        "vm": v_h.astype(BF16),
        "wq": Wq.astype(BF16),
        "wo": Wo.astype(BF16),
        "bqs": (bq * SCALE).astype(np.float32),
        "boe": bo.astype(np.float32),
        "onesf": np.ones((128, 1), dtype=np.float32),
    }
    in_maps = []
    for c in range(NCORE):
        xs = model_embed[BPC * c : BPC * (c + 1)].reshape(TOK, D)
        m = dict(shared)
        m["xT"] = np.ascontiguousarray(xs.T).astype(BF16)
        in_maps.append(m)
    return in_maps


def kernel(**inputs):
    nc = _get_nc()
    in_maps = _make_in_maps(**inputs)
    res = bass_utils.run_bass_kernel_spmd(nc, in_maps, core_ids=list(range(NCORE)))

    out = np.empty((B, T, P), dtype=np.float32)
    for c in range(NCORE):
        outT_c = res.results[c]["outT"]  # [P, TOK]
        out[BPC * c : BPC * (c + 1)] = outT_c.T.reshape(BPC, T, P)
    return out
